# revision 50
# baseline (speedup 1.0000x reference)
"""Trainium2 Bass kernel for AttentionWithRelPos.

Reference computation (fp32):
    qkv = x @ w_qkv.T                      # [B, N, 3C]
    q, k, v = split/reshape                # [B, H, N, HD]
    attn = softmax(q @ k.T * scale + bias) # bias gathered from rel_pos
    out  = (attn @ v).merge_heads @ w_proj.T + b_proj

Sharding: data-parallel over batch across 8 NeuronCores (8 batches/core).
All matmuls in bf16 with fp32 PSUM accumulation.

v2 design — transposed-S attention with fused softmax denominator:
  1. qkT = WqkT-stationary @ xT            -> [1536, 1576]  (q rows pre-scaled)
  2. v   = xT-stationary @ WvT             -> per-(batch,ktile) [rows, 12, 65]
     with a constant 1.0 in column 64 of each head slot (memset once).
  3. per (batch-pair bp, head h):
     S^T[k, q] for both batches side by side in ONE PSUM bank [kn, 2*197]
     (2 matmuls per k-tile, kT-stationary, qT moving), rel-pos bias^T
     accumulated by ONE identity-stationary matmul against the host-side
     duplicated bias table. No max subtraction (logits are O(1)): one ACT
     exp per k-tile evacuates PSUM -> SBUF bf16 P^T directly.
     PV with the ones-column: out[q, 0:64] = attn-out, out[q, 64] = rowsum.
     Per-partition reciprocal + scale on evacuation [q, 64] (Pool), then a
     single small PE transpose [q,64]->[64,q] per q-tile into attT.
  4. y = attT-stationary @ WpT             -> [1576, 768] -> DRAM
Emission is diagonal-wave interleaved so qk-proj, v-proj, attention, and
proj overlap. Host adds b_proj and re-assembles [64, 197, 768].
"""

import sys

if "/opt/trn_rl_repo" not in sys.path:
    sys.path.insert(0, "/opt/trn_rl_repo")

import numpy as np
import ml_dtypes

BF16 = ml_dtypes.bfloat16

B, DIM, HEADS, N = 64, 768, 12, 197
HD = DIM // HEADS  # 64
SCALE = HD ** -0.5
NCORES = 8
BL = B // NCORES  # 8 batches per core
KC = DIM // 128  # 6 contraction chunks

_CACHE = {}


def _build(bl=BL, lb=3, lc=4, s_bufs=2, o_bufs=4):
    """Build + compile the per-core Bass program. Returns the compiled nc."""
    import concourse.bacc as bacc
    import concourse.bass as bass
    import concourse.tile as tile
    from concourse import mybir
    from contextlib import ExitStack

    assert bl % 2 == 0, "batch pairing requires even bl"
    f32 = mybir.dt.float32
    bf16 = mybir.dt.bfloat16
    ACTF = mybir.ActivationFunctionType

    tok = bl * N
    NBP = bl // 2  # batch pairs

    nc = bacc.Bacc("TRN2", target_bir_lowering=False, debug=False,
                   enable_asserts=False, num_devices=NCORES)

    xT = nc.dram_tensor("xT", (DIM, tok), bf16, kind="ExternalInput").ap()
    wqkT = nc.dram_tensor("wqkT", (DIM, 2 * DIM), bf16, kind="ExternalInput").ap()
    wvT = nc.dram_tensor("wvT", (DIM, DIM), bf16, kind="ExternalInput").ap()
    wpT = nc.dram_tensor("wpT", (DIM, DIM), bf16, kind="ExternalInput").ap()
    biasT2 = nc.dram_tensor("biasT2", (HEADS, N, N), bf16,
                            kind="ExternalInput").ap()
    ident = nc.dram_tensor("ident", (128, 128), bf16, kind="ExternalInput").ap()
    y = nc.dram_tensor("y", (tok, DIM), f32, kind="ExternalOutput").ap()

    # token-chunking for qk-proj moving dims
    NCH = 4 if tok % 4 == 0 else 1
    CH = tok // NCH  # 394 for bl=8
    assert CH <= 512
    mt_sizes = [128] * (tok // 128) + ([tok % 128] if tok % 128 else [])
    qt_sizes = [128, N - 128]

    with ExitStack() as ctx:
        tc = ctx.enter_context(tile.TileContext(nc))
        singles = ctx.enter_context(tc.tile_pool(name="singles", bufs=1))
        mm_psum = ctx.enter_context(tc.tile_pool(name="mm_psum", bufs=2, space="PSUM"))
        s_psum = ctx.enter_context(tc.tile_pool(name="s_psum", bufs=s_bufs, space="PSUM"))
        # o-tiles and transpose tiles share one 4-bank ring (tag "o")
        o_psum = ctx.enter_context(tc.tile_pool(name="o_psum", bufs=o_bufs, space="PSUM"))
        work = ctx.enter_context(tc.tile_pool(name="work", bufs=5))
        stats = ctx.enter_context(tc.tile_pool(name="stats", bufs=12))

        # ---- persistent SBUF tensors ----
        xT_sb = singles.tile([128, KC, tok], bf16)
        wqk_sb = singles.tile([128, KC, 2 * DIM], bf16)
        wv_sb = singles.tile([128, KC, DIM], bf16)
        wp_sb = singles.tile([128, KC, DIM], bf16)
        biasT_sb = singles.tile([128, HEADS, 2, 2 * N], bf16)
        id_sb = singles.tile([128, 128], bf16)
        qkT_sb = singles.tile([128, 2 * KC, tok], bf16)
        vp_sb = singles.tile([128, bl, 2, HEADS, 65], bf16)
        attT_sb = singles.tile([128, KC, tok], bf16)

        # ---- input DMAs ----
        # The cost model serializes all transfers on one DMA lane (~360GB/s)
        # and charges ~630ns of serialized issue per DMA, so: one queue,
        # strict consumption order, few batched DMAs.
        # wqkT is column-interleaved on host: pair p occupies cols
        # [256p:256p+256] = [q-chunk p | k-chunk p], so the pair-0 piece the
        # first attention heads need is one contiguous 256-col block per kc.
        dma = lambda out, in_: nc.sync.dma_start(out=out, in_=in_)

        # the pair-0 wqk piece (one DMA), then the front half of xT per kc
        # (qkproj pair 0, n-chunks 0-1, vproj pairs 0-1)
        HT = tok // 2 if NCH == 4 else tok
        dma(wqk_sb[:, :, 0:256],
            wqkT[:, 0:256].rearrange("(c p) n -> p c n", p=128))
        for kc in range(KC):
            dma(xT_sb[:, kc, 0:HT], xT[kc * 128:(kc + 1) * 128, 0:HT])
        # bias for h0/h1 (first S units) + ident (first transposes)
        dma(biasT_sb[:, 0:2, 0, 0:N],
            biasT2[0:2, 0:128, :].rearrange("h p n -> p h n"))
        dma(id_sb[:, :], ident[:, :])
        dma(biasT_sb[0:N - 128, 0:2, 1, 0:N],
            biasT2[0:2, 128:N, :].rearrange("h p n -> p h n"))
        # wv in two 3-chunk DMAs (vproj pair 0 runs while the rest streams)
        for half in range(2):
            r0 = half * 384
            dma(wv_sb[:, 3 * half:3 * half + 3, :],
                wvT[r0:r0 + 384, :].rearrange("(c p) n -> p c n", p=128))
        # back half of xT (token batches 4-7; vproj pair 2 starts at wave 2)
        if HT < tok:
            for half in range(2):
                r0 = half * 384
                dma(xT_sb[:, 3 * half:3 * half + 3, HT:tok],
                    xT[r0:r0 + 384, HT:tok].rearrange("(c p) n -> p c n", p=128))
        # remaining wqk pairs (consumed from wave 1 onward)
        for half in range(2):
            r0 = half * 384
            dma(wqk_sb[:, 3 * half:3 * half + 3, 256:2 * DIM],
                wqkT[r0:r0 + 384, 256:2 * DIM].rearrange("(c p) n -> p c n", p=128))
        # remaining bias heads (exp'd then raw)
        dma(biasT_sb[:, 2:HEADS, 0, 0:N],
            biasT2[2:HEADS, 0:128, :].rearrange("h p n -> p h n"))
        dma(biasT_sb[0:N - 128, 2:HEADS, 1, 0:N],
            biasT2[2:HEADS, 128:N, :].rearrange("h p n -> p h n"))
        # wp last (proj starts ~150us in)
        dma(wp_sb[:, :, :], wpT[:, :].rearrange("(c p) n -> p c n", p=128))

        # duplicate the bias tables on-device (halves their DMA bytes):
        # h0/h1 first so A(0,0) unblocks, then the rest in big copies.
        # The raw table is duplicated by the otherwise-idle-at-startup Pool.
        for kt in range(2):
            kn = qt_sizes[kt]
            nc.vector.tensor_copy(biasT_sb[0:kn, 0:2, kt, N:2 * N],
                                  biasT_sb[0:kn, 0:2, kt, 0:N])
        for kt in range(2):
            kn = qt_sizes[kt]
            nc.vector.tensor_copy(biasT_sb[0:kn, 2:HEADS, kt, N:2 * N],
                                  biasT_sb[0:kn, 2:HEADS, kt, 0:N])
        # ones column of every (b, kt, h) v slot
        for b in range(bl):
            nc.vector.memset(vp_sb[:, b, :, :, 64:65], 1.0)

        def emit_qkproj(m, npair, pingpong=False):
            # wqk columns are host-interleaved: q-chunk m at 256m, k-chunk
            # (m-KC) at 256(m-KC)+128. One call does n-chunks 2*npair and
            # 2*npair+1. pingpong interleaves the two n-chunks' kc loops
            # (halves the PE stall while xT chunks are still arriving —
            # wave-0 only: it serializes the bank ring otherwise).
            c0 = 256 * m if m < KC else 256 * (m - KC) + 128
            ns = [2 * npair, 2 * npair + 1] if NCH == 4 else [0]
            if pingpong:
                pss = [mm_psum.tile([128, 512], f32, tag="mm", name="ps")
                       for _ in ns]
                for kc in range(KC):
                    for ps, n in zip(pss, ns):
                        nc.tensor.matmul(
                            ps[:, 0:CH],
                            lhsT=wqk_sb[:, kc, c0:c0 + 128],
                            rhs=xT_sb[:, kc, n * CH:(n + 1) * CH],
                            start=(kc == 0), stop=(kc == KC - 1),
                        )
                for ps, n in zip(pss, ns):
                    dst = qkT_sb[:, m, n * CH:(n + 1) * CH]
                    nc.scalar.copy(out=dst, in_=ps[:, 0:CH])
            else:
                for n in ns:
                    ps = mm_psum.tile([128, 512], f32, tag="mm", name="ps")
                    for kc in range(KC):
                        nc.tensor.matmul(
                            ps[:, 0:CH],
                            lhsT=wqk_sb[:, kc, c0:c0 + 128],
                            rhs=xT_sb[:, kc, n * CH:(n + 1) * CH],
                            start=(kc == 0), stop=(kc == KC - 1),
                        )
                    dst = qkT_sb[:, m, n * CH:(n + 1) * CH]
                    nc.scalar.copy(out=dst, in_=ps[:, 0:CH])

        def emit_vproj(b):
            for kt in range(2):
                rows = qt_sizes[kt]
                t0 = b * N + kt * 128
                for n2 in range(2):
                    ps = mm_psum.tile([128, 6, 64], f32, tag="mm", name="vps")
                    for kc in range(KC):
                        nc.tensor.matmul(
                            ps[0:rows, :, :],
                            lhsT=xT_sb[:, kc, t0:t0 + rows],
                            rhs=wv_sb[:, kc, n2 * 384:(n2 + 1) * 384],
                            start=(kc == 0), stop=(kc == KC - 1),
                        )
                    dst = vp_sb[0:rows, b, kt, n2 * 6:(n2 + 1) * 6, 0:64]
                    nc.vector.tensor_copy(dst, ps[0:rows, :, :])

        def emit_S(bp, h):
            """S^T + bias + exp for both batches of the pair; returns pT.

            The rel-pos bias is folded in by the otherwise-idle Pool engine
            as a multiply with the host-side exp(bias^T) table:
            exp(S+B) = exp(S)*exp(B).
            """
            b0 = 2 * bp
            mq = h // 2
            mk = KC + h // 2
            po = (h % 2) * 64

            # P^T for (kt, bi): [kn, 197] each, kt-major then (bi, q) packed
            pT = work.tile([128, 2, 2 * N], bf16, tag="pT", name="pT", bufs=6)
            for kt in range(2):
                kn = qt_sizes[kt]
                kT = qkT_sb[po:po + 64, mk, :]
                s_ps = s_psum.tile([128, 2 * N], f32, tag="s", name="s_ps")
                for bi in range(2):
                    t0 = (b0 + bi) * N
                    # bi=1 writes bytes still pending-zero from bi=0's bank
                    # start, so start=False yields a fresh write there.
                    nc.tensor.matmul(
                        s_ps[0:kn, bi * N:(bi + 1) * N],
                        lhsT=kT[:, t0 + kt * 128:t0 + kt * 128 + kn],
                        rhs=qkT_sb[po:po + 64, mq, t0:t0 + N],
                        start=(bi == 0), stop=(bi == 1),
                    )
                pexp = work.tile([128, 2 * N], bf16, tag="pexp",
                                 name="pexp", bufs=6)
                nc.scalar.activation(
                    out=pexp[0:kn, :],
                    in_=s_ps[0:kn, :],
                    func=ACTF.Exp,
                )
                nc.gpsimd.tensor_mul(
                    pT[0:kn, kt, :], pexp[0:kn, :],
                    biasT_sb[0:kn, h, kt, :]
                )
            return pT

        def emit_B(bp, h, pT):
            """PV matmuls + per-partition normalize for both batches."""
            b0 = 2 * bp
            atts = []
            for bi in range(2):
                b = b0 + bi
                att = work.tile([128, 2, 64], bf16, tag="att", name="att", bufs=8)
                for qt in range(2):
                    qn = qt_sizes[qt]
                    o_ps = o_psum.tile([128, 65], f32, tag="o", name="o_ps")
                    for kt in range(2):
                        kn = qt_sizes[kt]
                        nc.tensor.matmul(
                            o_ps[0:qn, :],
                            lhsT=pT[0:kn, kt, bi * N + qt * 128:
                                    bi * N + qt * 128 + qn],
                            rhs=vp_sb[0:kn, b, kt, h, :],
                            start=(kt == 0), stop=(kt == 1),
                        )
                    rcp = stats.tile([128, 1], f32, tag="rcp")
                    nc.vector.reciprocal(rcp[0:qn, :], o_ps[0:qn, 64:65])
                    nc.vector.tensor_scalar_mul(
                        att[0:qn, qt, :], o_ps[0:qn, 0:64], rcp[0:qn, :]
                    )
                atts.append(att)
            return atts

        def emit_C(bp, h, atts):
            """Transpose att -> attT rows for this head, both batches."""
            b0 = 2 * bp
            mq = h // 2
            po = (h % 2) * 64
            for bi in range(2):
                b = b0 + bi
                att = atts[bi]
                tr = o_psum.tile([64, 256], bf16, tag="o", name="tr")
                for qt in range(2):
                    qn = qt_sizes[qt]
                    nc.tensor.transpose(
                        tr[:, qt * 128:qt * 128 + qn],
                        in_=att[0:qn, qt, :],
                        identity=id_sb[0:qn, 0:qn],
                    )
                dst = attT_sb[po:po + 64, mq, b * N:(b + 1) * N]
                nc.vector.tensor_copy(dst, tr[:, 0:N])

        def emit_proj(mt):
            rows = mt_sizes[mt]
            t0 = mt * 128
            for n2 in range(2):
                ps = mm_psum.tile([128, 512], f32, tag="mm", name="ps")
                for kc in range(KC):
                    nc.tensor.matmul(
                        ps[0:rows, 0:384],
                        lhsT=attT_sb[:, kc, t0:t0 + rows],
                        rhs=wp_sb[:, kc, n2 * 384:(n2 + 1) * 384],
                        start=(kc == 0), stop=(kc == KC - 1),
                    )
                yst = work.tile([128, 384], f32, tag="yst")
                nc.scalar.copy(out=yst[0:rows, :], in_=ps[0:rows, 0:384])
                nc.sync.dma_start(
                    out=y[t0:t0 + rows, n2 * 384:(n2 + 1) * 384],
                    in_=yst[0:rows, :],
                )

        # ---- emission: diagonal wave over (batch-pair, head), software
        # pipelined three stages deep: per step emit A_i (S+exp), then
        # C_{i-2} (transpose), then B_{i-1} (PV+normalize). The two-unit
        # A->B and B->C lags hide the cross-engine exp / normalize latency
        # behind PE work of neighboring units. ----
        proj_ptr = [0]
        done_pairs = [0]

        def emit_proj_upto(limit):
            limit = min(limit, len(mt_sizes))
            while proj_ptr[0] < limit:
                emit_proj(proj_ptr[0])
                proj_ptr[0] += 1

        # software pipeline: at step i emit A_i, B_{i-LB}, C_{i-LC}
        LB, LC = lb, lc
        a_done = []   # (bp, h, pT) in unit order
        b_done = []   # (bp, h, atts) in unit order
        nb = [0]      # units with B emitted
        ncc = [0]     # units with C emitted

        def note_C(bp, h):
            if h == HEADS - 1:
                done_pairs[0] += 1
                emit_proj_upto(done_pairs[0] * 2 * N // 128)

        def pump(i):
            # emit B for unit i-LB and C for unit i-LC (i = index of the A
            # just emitted; i == None during the final drain)
            if nb[0] < len(a_done) and (i is None or nb[0] <= i - LB):
                bp_, h_, pT_ = a_done[nb[0]]
                b_done.append((bp_, h_, emit_B(bp_, h_, pT_)))
                a_done[nb[0]] = None
                nb[0] += 1
            if ncc[0] < len(b_done) and (i is None or ncc[0] <= i - LC):
                cbp, ch, atts = b_done[ncc[0]]
                emit_C(cbp, ch, atts)
                note_C(cbp, ch)
                b_done[ncc[0]] = None
                ncc[0] += 1

        def step(bp, h):
            a_done.append((bp, h, emit_S(bp, h)))
            pump(len(a_done) - 1)

        for w in range(NBP + HEADS - 1):
            # qk chunk-pair 0 front-half first (its weights land first);
            # later pieces are emitted just-in-time at wave ends (below) so
            # still-in-flight DMAs never stall runnable attention work.
            # Front halves (token batches 0-3, n-chunks 0-1) serve attention
            # of pairs 0-1; back halves (n-chunks 2-3) serve pairs 2-3 whose
            # first S of chunk m is at wave 2m+2.
            if w == 0:
                emit_qkproj(0, 0, pingpong=True)
                emit_qkproj(KC, 0, pingpong=True)
            for bp in range(NBP):
                h = w - bp
                if 0 <= h < HEADS:
                    step(bp, h)
            # v-proj for pair w after its A(w, 0) but before its first PV
            # (emitted at wave w+1), so the slow wv DMA doesn't stall
            # already-runnable attention matmuls.
            if w < NBP:
                emit_vproj(2 * w)
                emit_vproj(2 * w + 1)
            if w % 2 == 1:
                m = (w + 1) // 2
                if m < KC:
                    emit_qkproj(m, 0)
                    emit_qkproj(KC + m, 0)
                if NCH == 4:
                    mb = (w - 1) // 2
                    if mb < KC:
                        emit_qkproj(mb, 1)
                        emit_qkproj(KC + mb, 1)
        # drain the pipeline
        total_units = NBP * HEADS
        while ncc[0] < total_units:
            pump(None)
        emit_proj_upto(len(mt_sizes))

    nc.compile()
    return nc


def _prep_shared(w_qkv, w_proj, rel_pos, rel_pos_index):
    """Host-side input prep shared across cores (weights / bias / identity)."""
    w_qkv = np.asarray(w_qkv, dtype=np.float32)
    w_proj = np.asarray(w_proj, dtype=np.float32)
    rel_pos = np.asarray(rel_pos, dtype=np.float32)
    rel_pos_index = np.asarray(rel_pos_index)

    wqk = w_qkv[:2 * DIM].copy()
    wqk[:DIM] *= SCALE  # fold attention scale into Wq
    wqkT = np.ascontiguousarray(wqk.T)  # [DIM, 2*DIM]
    # interleave q/k column chunks: pair p -> cols [256p:256p+128]=q-chunk p,
    # [256p+128:256(p+1)]=k-chunk p
    wqkT = wqkT.reshape(DIM, 2, KC, 128).transpose(0, 2, 1, 3)
    wqkT = np.ascontiguousarray(wqkT.reshape(DIM, 2 * DIM)).astype(BF16)
    wvT = np.ascontiguousarray(w_qkv[2 * DIM:].T).astype(BF16)
    wpT = np.ascontiguousarray(w_proj.T).astype(BF16)

    bias_full = np.zeros((HEADS, N, N), dtype=np.float32)
    bias_full[:, 1:, 1:] = rel_pos[:, rel_pos_index]
    # exp of the transposed bias ([k, q]) — folded into P by a multiply on
    # device (exp(S+B) = exp(S)*exp(B)); duplication along q happens on-device
    biasT2 = np.ascontiguousarray(np.exp(np.swapaxes(bias_full, 1, 2))).astype(BF16)

    ident = np.eye(128, dtype=BF16)
    return {"wqkT": wqkT, "wvT": wvT, "wpT": wpT, "biasT2": biasT2,
            "ident": ident}


def _prep_core(x, core, bl=BL):
    """Per-core xT: [DIM, bl*N] bf16."""
    xc = np.asarray(x[core * bl:(core + 1) * bl], dtype=np.float32)
    xT = np.ascontiguousarray(xc.reshape(bl * N, DIM).T).astype(BF16)
    return xT


def kernel(x, w_qkv, w_proj, b_proj, rel_pos, rel_pos_index):
    from concourse.bass_utils import run_bass_kernel_spmd

    x = np.asarray(x, dtype=np.float32)
    w_qkv = np.asarray(w_qkv, dtype=np.float32)
    w_proj = np.asarray(w_proj, dtype=np.float32)
    b_proj = np.asarray(b_proj, dtype=np.float32)
    rel_pos = np.asarray(rel_pos, dtype=np.float32)
    rel_pos_index = np.asarray(rel_pos_index)

    if "nc" not in _CACHE:
        _CACHE["nc"] = _build(BL)
    nc = _CACHE["nc"]

    shared = _prep_shared(w_qkv, w_proj, rel_pos, rel_pos_index)
    in_maps = []
    for core in range(NCORES):
        m = dict(shared)
        m["xT"] = _prep_core(x, core)
        in_maps.append(m)

    try:
        y_cores = _run_cached(nc, in_maps)
    except Exception:
        res = run_bass_kernel_spmd(nc, in_maps, core_ids=list(range(NCORES)))
        y_cores = [r["y"] for r in res.results]
    y = np.concatenate(
        [yc.reshape(BL, N, DIM) for yc in y_cores], axis=0
    ).astype(np.float32)
    return y + b_proj[None, None, :]


def _run_cached(nc, in_maps):
    """Execute via a cached jitted shard_map executable (run_bass_kernel_spmd
    re-traces per call; this path pays tracing/lowering only once)."""
    import jax
    from jax.sharding import Mesh, PartitionSpec, NamedSharding
    from jax.experimental.shard_map import shard_map
    from concourse import bass2jax, mybir

    if "exe" not in _CACHE:
        bass2jax.install_neuronx_cc_hook()
        pname = nc.partition_id_tensor.name if nc.partition_id_tensor else None
        in_names, out_names, out_avals, zeros = [], [], [], []
        for alloc in nc.m.functions[0].allocations:
            if not isinstance(alloc, mybir.MemoryLocationSet):
                continue
            name = alloc.memorylocations[0].name
            if alloc.kind == "ExternalInput":
                if name != pname:
                    in_names.append(name)
            elif alloc.kind == "ExternalOutput":
                out_names.append(name)
                shape = tuple(alloc.tensor_shape)
                dtype = mybir.dt.np(alloc.dtype)
                out_avals.append(jax.core.ShapedArray(shape, dtype))
                zeros.append(np.zeros(shape, dtype))
        n_params = len(in_names)
        all_in = in_names + out_names + ([pname] if pname else [])

        def _body(*args):
            operands = list(args)
            if pname is not None:
                operands.append(bass2jax.partition_id_tensor())
            return tuple(bass2jax._bass_exec_p.bind(
                *operands, out_avals=tuple(out_avals), in_names=tuple(all_in),
                out_names=tuple(out_names), lowering_input_output_aliases=(),
                sim_require_finite=True, sim_require_nnan=True, nc=nc))

        devices = jax.devices()[:NCORES]
        mesh = Mesh(np.asarray(devices), ("core",))
        n_outs = len(out_names)
        sharded = jax.jit(
            shard_map(_body, mesh=mesh,
                      in_specs=(PartitionSpec("core"),) * (n_params + n_outs),
                      out_specs=(PartitionSpec("core"),) * n_outs,
                      check_rep=False),
            keep_unused=True,
        )
        sh = NamedSharding(mesh, PartitionSpec("core"))
        zero_dev = [
            jax.device_put(
                np.zeros((NCORES * z.shape[0], *z.shape[1:]), z.dtype), sh)
            for z in zeros
        ]
        _CACHE["exe"] = (sharded, in_names, out_names, zero_dev, sh)

    sharded, in_names, out_names, zero_dev, sh = _CACHE["exe"]
    concat_in = [
        np.concatenate([np.asarray(in_maps[c][nm]) for c in range(NCORES)],
                       axis=0)
        for nm in in_names
    ]
    out = sharded(*[jax.device_put(a, sh) for a in concat_in], *zero_dev)
    yi = out_names.index("y")
    y_all = np.asarray(out[yi])
    rows = y_all.shape[0] // NCORES
    return [y_all[c * rows:(c + 1) * rows] for c in range(NCORES)]


# revision 53
# speedup vs baseline: 3.4073x; 3.4073x over previous
"""Trainium2 Bass kernel for AttentionWithRelPos.

Reference computation (fp32):
    qkv = x @ w_qkv.T                      # [B, N, 3C]
    q, k, v = split/reshape                # [B, H, N, HD]
    attn = softmax(q @ k.T * scale + bias) # bias gathered from rel_pos
    out  = (attn @ v).merge_heads @ w_proj.T + b_proj

Sharding: data-parallel over batch across 8 NeuronCores (8 batches/core).
All matmuls in bf16 with fp32 PSUM accumulation.

Design — transposed-S attention with fused softmax denominator:
  1. qkT = WqkT-stationary @ xT            -> [1536, 1576]  (q rows pre-scaled)
  2. v   = xT-stationary @ WvT             -> per-(batch,ktile) [rows, 12, 65]
     with a constant 1.0 in column 64 of each head slot (memset once).
  3. per (batch-pair bp, head h):
     S^T[k, q] for both batches side by side in ONE PSUM bank [kn, 2*197]
     (2 matmuls per k-tile, kT-stationary, qT moving). No max subtraction
     (logits are O(1)): one ACT exp per k-tile evacuates PSUM -> SBUF bf16;
     the rel-pos bias is folded in by the otherwise-idle Pool engine as a
     multiply with the host-side exp(bias^T) table (exp(S+B)=exp(S)*exp(B)).
     PV with the ones-column: out[q, 0:64] = attn-out, out[q, 64] = rowsum.
     Per-partition reciprocal + scale on evacuation [q, 64] (DVE), then a
     single small PE transpose [q,64]->[64,q] per q-tile into attT.
  4. y = attT-stationary @ WpT             -> [1576, 768] -> DRAM
Emission is a diagonal wave over (batch-pair, head) units, software-
pipelined three stages deep (S+exp / PV+normalize / transpose+evacuate) so
the cross-engine latencies hide behind PE work of neighboring units, with
qk-proj, v-proj and proj streamed just-in-time around it. Input DMAs are
ordered/batched to match consumption. Host adds b_proj and re-assembles
[64, 197, 768].
"""

import sys

if "/opt/trn_rl_repo" not in sys.path:
    sys.path.insert(0, "/opt/trn_rl_repo")

import numpy as np
import ml_dtypes

BF16 = ml_dtypes.bfloat16

B, DIM, HEADS, N = 64, 768, 12, 197
HD = DIM // HEADS  # 64
SCALE = HD ** -0.5
NCORES = 8
BL = B // NCORES  # 8 batches per core
KC = DIM // 128  # 6 contraction chunks

_CACHE = {}


def _build(bl=BL, lb=3, lc=4, s_bufs=2, o_bufs=4):
    """Build + compile the per-core Bass program. Returns the compiled nc."""
    import concourse.bacc as bacc
    import concourse.bass as bass
    import concourse.tile as tile
    from concourse import mybir
    from contextlib import ExitStack

    assert bl % 2 == 0, "batch pairing requires even bl"
    f32 = mybir.dt.float32
    bf16 = mybir.dt.bfloat16
    ACTF = mybir.ActivationFunctionType

    tok = bl * N
    NBP = bl // 2  # batch pairs

    nc = bacc.Bacc("TRN2", target_bir_lowering=False, debug=False,
                   enable_asserts=False, num_devices=NCORES)

    xT = nc.dram_tensor("xT", (DIM, tok), bf16, kind="ExternalInput").ap()
    wqkT = nc.dram_tensor("wqkT", (DIM, 2 * DIM), bf16, kind="ExternalInput").ap()
    wvT = nc.dram_tensor("wvT", (DIM, DIM), bf16, kind="ExternalInput").ap()
    wpT = nc.dram_tensor("wpT", (DIM, DIM), bf16, kind="ExternalInput").ap()
    biasT2 = nc.dram_tensor("biasT2", (HEADS, N, N), bf16,
                            kind="ExternalInput").ap()
    ident = nc.dram_tensor("ident", (128, 128), bf16, kind="ExternalInput").ap()
    y = nc.dram_tensor("y", (tok, DIM), f32, kind="ExternalOutput").ap()

    # token-chunking for qk-proj moving dims
    NCH = 4 if tok % 4 == 0 else 1
    CH = tok // NCH  # 394 for bl=8
    assert CH <= 512
    mt_sizes = [128] * (tok // 128) + ([tok % 128] if tok % 128 else [])
    qt_sizes = [128, N - 128]

    with ExitStack() as ctx:
        tc = ctx.enter_context(tile.TileContext(nc))
        singles = ctx.enter_context(tc.tile_pool(name="singles", bufs=1))
        mm_psum = ctx.enter_context(tc.tile_pool(name="mm_psum", bufs=2, space="PSUM"))
        s_psum = ctx.enter_context(tc.tile_pool(name="s_psum", bufs=s_bufs, space="PSUM"))
        # o-tiles and transpose tiles share one 4-bank ring (tag "o")
        o_psum = ctx.enter_context(tc.tile_pool(name="o_psum", bufs=o_bufs, space="PSUM"))
        work = ctx.enter_context(tc.tile_pool(name="work", bufs=5))
        stats = ctx.enter_context(tc.tile_pool(name="stats", bufs=12))

        # ---- persistent SBUF tensors ----
        xT_sb = singles.tile([128, KC, tok], bf16)
        wqk_sb = singles.tile([128, KC, 2 * DIM], bf16)
        wv_sb = singles.tile([128, KC, DIM], bf16)
        wp_sb = singles.tile([128, KC, DIM], bf16)
        biasT_sb = singles.tile([128, HEADS, 2, 2 * N], bf16)
        id_sb = singles.tile([128, 128], bf16)
        qkT_sb = singles.tile([128, 2 * KC, tok], bf16)
        vp_sb = singles.tile([128, bl, 2, HEADS, 65], bf16)
        attT_sb = singles.tile([128, KC, tok], bf16)

        # ---- input DMAs ----
        # The cost model serializes all transfers on one DMA lane (~360GB/s)
        # and charges ~630ns of serialized issue per DMA, so: one queue,
        # strict consumption order, few batched DMAs.
        # wqkT is column-interleaved on host: pair p occupies cols
        # [256p:256p+256] = [q-chunk p | k-chunk p], so the pair-0 piece the
        # first attention heads need is one contiguous 256-col block per kc.
        dma = lambda out, in_: nc.sync.dma_start(out=out, in_=in_)

        # the pair-0 wqk piece (one DMA), then the front half of xT per kc
        # (qkproj pair 0, n-chunks 0-1, vproj pairs 0-1)
        HT = tok // 2 if NCH == 4 else tok
        dma(wqk_sb[:, :, 0:256],
            wqkT[:, 0:256].rearrange("(c p) n -> p c n", p=128))
        for kc in range(KC):
            dma(xT_sb[:, kc, 0:HT], xT[kc * 128:(kc + 1) * 128, 0:HT])
        # bias for h0/h1 (first S units) + ident (first transposes)
        dma(biasT_sb[:, 0:2, 0, 0:N],
            biasT2[0:2, 0:128, :].rearrange("h p n -> p h n"))
        dma(id_sb[:, :], ident[:, :])
        dma(biasT_sb[0:N - 128, 0:2, 1, 0:N],
            biasT2[0:2, 128:N, :].rearrange("h p n -> p h n"))
        # wv in two 3-chunk DMAs (vproj pair 0 runs while the rest streams)
        for half in range(2):
            r0 = half * 384
            dma(wv_sb[:, 3 * half:3 * half + 3, :],
                wvT[r0:r0 + 384, :].rearrange("(c p) n -> p c n", p=128))
        # back half of xT (token batches 4-7; vproj pair 2 starts at wave 2)
        if HT < tok:
            for half in range(2):
                r0 = half * 384
                dma(xT_sb[:, 3 * half:3 * half + 3, HT:tok],
                    xT[r0:r0 + 384, HT:tok].rearrange("(c p) n -> p c n", p=128))
        # remaining wqk pairs (consumed from wave 1 onward)
        for half in range(2):
            r0 = half * 384
            dma(wqk_sb[:, 3 * half:3 * half + 3, 256:2 * DIM],
                wqkT[r0:r0 + 384, 256:2 * DIM].rearrange("(c p) n -> p c n", p=128))
        # remaining bias heads
        dma(biasT_sb[:, 2:HEADS, 0, 0:N],
            biasT2[2:HEADS, 0:128, :].rearrange("h p n -> p h n"))
        dma(biasT_sb[0:N - 128, 2:HEADS, 1, 0:N],
            biasT2[2:HEADS, 128:N, :].rearrange("h p n -> p h n"))
        # wp last (proj starts ~150us in)
        dma(wp_sb[:, :, :], wpT[:, :].rearrange("(c p) n -> p c n", p=128))

        # duplicate the bias table on-device (halves its DMA bytes):
        # h0/h1 first so A(0,0) unblocks, then the rest in big copies.
        for kt in range(2):
            kn = qt_sizes[kt]
            nc.vector.tensor_copy(biasT_sb[0:kn, 0:2, kt, N:2 * N],
                                  biasT_sb[0:kn, 0:2, kt, 0:N])
        for kt in range(2):
            kn = qt_sizes[kt]
            nc.vector.tensor_copy(biasT_sb[0:kn, 2:HEADS, kt, N:2 * N],
                                  biasT_sb[0:kn, 2:HEADS, kt, 0:N])
        # ones column of every (b, kt, h) v slot
        for b in range(bl):
            nc.vector.memset(vp_sb[:, b, :, :, 64:65], 1.0)

        def emit_qkproj(m, npair, pingpong=False):
            # wqk columns are host-interleaved: q-chunk m at 256m, k-chunk
            # (m-KC) at 256(m-KC)+128. One call does n-chunks 2*npair and
            # 2*npair+1. pingpong interleaves the two n-chunks' kc loops
            # (halves the PE stall while xT chunks are still arriving —
            # wave-0 only: it serializes the bank ring otherwise).
            c0 = 256 * m if m < KC else 256 * (m - KC) + 128
            ns = [2 * npair, 2 * npair + 1] if NCH == 4 else [0]
            if pingpong:
                pss = [mm_psum.tile([128, 512], f32, tag="mm", name="ps")
                       for _ in ns]
                for kc in range(KC):
                    for ps, n in zip(pss, ns):
                        nc.tensor.matmul(
                            ps[:, 0:CH],
                            lhsT=wqk_sb[:, kc, c0:c0 + 128],
                            rhs=xT_sb[:, kc, n * CH:(n + 1) * CH],
                            start=(kc == 0), stop=(kc == KC - 1),
                        )
                for ps, n in zip(pss, ns):
                    dst = qkT_sb[:, m, n * CH:(n + 1) * CH]
                    nc.scalar.copy(out=dst, in_=ps[:, 0:CH])
            else:
                for n in ns:
                    ps = mm_psum.tile([128, 512], f32, tag="mm", name="ps")
                    for kc in range(KC):
                        nc.tensor.matmul(
                            ps[:, 0:CH],
                            lhsT=wqk_sb[:, kc, c0:c0 + 128],
                            rhs=xT_sb[:, kc, n * CH:(n + 1) * CH],
                            start=(kc == 0), stop=(kc == KC - 1),
                        )
                    dst = qkT_sb[:, m, n * CH:(n + 1) * CH]
                    nc.scalar.copy(out=dst, in_=ps[:, 0:CH])

        def emit_vproj(b):
            for kt in range(2):
                rows = qt_sizes[kt]
                t0 = b * N + kt * 128
                for n2 in range(2):
                    ps = mm_psum.tile([128, 6, 64], f32, tag="mm", name="vps")
                    for kc in range(KC):
                        nc.tensor.matmul(
                            ps[0:rows, :, :],
                            lhsT=xT_sb[:, kc, t0:t0 + rows],
                            rhs=wv_sb[:, kc, n2 * 384:(n2 + 1) * 384],
                            start=(kc == 0), stop=(kc == KC - 1),
                        )
                    dst = vp_sb[0:rows, b, kt, n2 * 6:(n2 + 1) * 6, 0:64]
                    nc.vector.tensor_copy(dst, ps[0:rows, :, :])

        def emit_S(bp, h):
            """S^T + bias + exp for both batches of the pair; returns pT.

            The rel-pos bias is folded in by the otherwise-idle Pool engine
            as a multiply with the host-side exp(bias^T) table:
            exp(S+B) = exp(S)*exp(B).
            """
            b0 = 2 * bp
            mq = h // 2
            mk = KC + h // 2
            po = (h % 2) * 64

            # P^T for (kt, bi): [kn, 197] each, kt-major then (bi, q) packed
            pT = work.tile([128, 2, 2 * N], bf16, tag="pT", name="pT", bufs=6)
            for kt in range(2):
                kn = qt_sizes[kt]
                kT = qkT_sb[po:po + 64, mk, :]
                s_ps = s_psum.tile([128, 2 * N], f32, tag="s", name="s_ps")
                for bi in range(2):
                    t0 = (b0 + bi) * N
                    # bi=1 writes bytes still pending-zero from bi=0's bank
                    # start, so start=False yields a fresh write there.
                    nc.tensor.matmul(
                        s_ps[0:kn, bi * N:(bi + 1) * N],
                        lhsT=kT[:, t0 + kt * 128:t0 + kt * 128 + kn],
                        rhs=qkT_sb[po:po + 64, mq, t0:t0 + N],
                        start=(bi == 0), stop=(bi == 1),
                    )
                pexp = work.tile([128, 2 * N], bf16, tag="pexp",
                                 name="pexp", bufs=6)
                nc.scalar.activation(
                    out=pexp[0:kn, :],
                    in_=s_ps[0:kn, :],
                    func=ACTF.Exp,
                )
                nc.gpsimd.tensor_mul(
                    pT[0:kn, kt, :], pexp[0:kn, :],
                    biasT_sb[0:kn, h, kt, :]
                )
            return pT

        def emit_B(bp, h, pT):
            """PV matmuls + per-partition normalize for both batches."""
            b0 = 2 * bp
            atts = []
            for bi in range(2):
                b = b0 + bi
                att = work.tile([128, 2, 64], bf16, tag="att", name="att", bufs=8)
                for qt in range(2):
                    qn = qt_sizes[qt]
                    o_ps = o_psum.tile([128, 65], f32, tag="o", name="o_ps")
                    for kt in range(2):
                        kn = qt_sizes[kt]
                        nc.tensor.matmul(
                            o_ps[0:qn, :],
                            lhsT=pT[0:kn, kt, bi * N + qt * 128:
                                    bi * N + qt * 128 + qn],
                            rhs=vp_sb[0:kn, b, kt, h, :],
                            start=(kt == 0), stop=(kt == 1),
                        )
                    rcp = stats.tile([128, 1], f32, tag="rcp")
                    nc.vector.reciprocal(rcp[0:qn, :], o_ps[0:qn, 64:65])
                    nc.vector.tensor_scalar_mul(
                        att[0:qn, qt, :], o_ps[0:qn, 0:64], rcp[0:qn, :]
                    )
                atts.append(att)
            return atts

        def emit_C(bp, h, atts):
            """Transpose att -> attT rows for this head, both batches."""
            b0 = 2 * bp
            mq = h // 2
            po = (h % 2) * 64
            for bi in range(2):
                b = b0 + bi
                att = atts[bi]
                tr = o_psum.tile([64, 256], bf16, tag="o", name="tr")
                for qt in range(2):
                    qn = qt_sizes[qt]
                    nc.tensor.transpose(
                        tr[:, qt * 128:qt * 128 + qn],
                        in_=att[0:qn, qt, :],
                        identity=id_sb[0:qn, 0:qn],
                    )
                dst = attT_sb[po:po + 64, mq, b * N:(b + 1) * N]
                nc.vector.tensor_copy(dst, tr[:, 0:N])

        def emit_proj(mt):
            rows = mt_sizes[mt]
            t0 = mt * 128
            for n2 in range(2):
                ps = mm_psum.tile([128, 512], f32, tag="mm", name="ps")
                for kc in range(KC):
                    nc.tensor.matmul(
                        ps[0:rows, 0:384],
                        lhsT=attT_sb[:, kc, t0:t0 + rows],
                        rhs=wp_sb[:, kc, n2 * 384:(n2 + 1) * 384],
                        start=(kc == 0), stop=(kc == KC - 1),
                    )
                yst = work.tile([128, 384], f32, tag="yst")
                nc.scalar.copy(out=yst[0:rows, :], in_=ps[0:rows, 0:384])
                nc.sync.dma_start(
                    out=y[t0:t0 + rows, n2 * 384:(n2 + 1) * 384],
                    in_=yst[0:rows, :],
                )

        # ---- emission: diagonal wave over (batch-pair, head), software
        # pipelined three stages deep: per step emit A_i (S+exp), then
        # C_{i-2} (transpose), then B_{i-1} (PV+normalize). The two-unit
        # A->B and B->C lags hide the cross-engine exp / normalize latency
        # behind PE work of neighboring units. ----
        proj_ptr = [0]
        done_pairs = [0]

        def emit_proj_upto(limit):
            limit = min(limit, len(mt_sizes))
            while proj_ptr[0] < limit:
                emit_proj(proj_ptr[0])
                proj_ptr[0] += 1

        # software pipeline: at step i emit A_i, B_{i-LB}, C_{i-LC}
        LB, LC = lb, lc
        a_done = []   # (bp, h, pT) in unit order
        b_done = []   # (bp, h, atts) in unit order
        nb = [0]      # units with B emitted
        ncc = [0]     # units with C emitted

        def note_C(bp, h):
            if h == HEADS - 1:
                done_pairs[0] += 1
                emit_proj_upto(done_pairs[0] * 2 * N // 128)

        def pump(i):
            # emit B for unit i-LB and C for unit i-LC (i = index of the A
            # just emitted; i == None during the final drain)
            if nb[0] < len(a_done) and (i is None or nb[0] <= i - LB):
                bp_, h_, pT_ = a_done[nb[0]]
                b_done.append((bp_, h_, emit_B(bp_, h_, pT_)))
                a_done[nb[0]] = None
                nb[0] += 1
            if ncc[0] < len(b_done) and (i is None or ncc[0] <= i - LC):
                cbp, ch, atts = b_done[ncc[0]]
                emit_C(cbp, ch, atts)
                note_C(cbp, ch)
                b_done[ncc[0]] = None
                ncc[0] += 1

        def step(bp, h):
            a_done.append((bp, h, emit_S(bp, h)))
            pump(len(a_done) - 1)

        for w in range(NBP + HEADS - 1):
            # qk chunk-pair 0 front-half first (its weights land first);
            # later pieces are emitted just-in-time at wave ends (below) so
            # still-in-flight DMAs never stall runnable attention work.
            # Front halves (token batches 0-3, n-chunks 0-1) serve attention
            # of pairs 0-1; back halves (n-chunks 2-3) serve pairs 2-3 whose
            # first S of chunk m is at wave 2m+2.
            if w == 0:
                emit_qkproj(0, 0, pingpong=True)
                emit_qkproj(KC, 0, pingpong=True)
            for bp in range(NBP):
                h = w - bp
                if 0 <= h < HEADS:
                    step(bp, h)
            # v-proj for pair w after its A(w, 0) but before its first PV
            # (emitted at wave w+1), so the slow wv DMA doesn't stall
            # already-runnable attention matmuls.
            if w < NBP:
                emit_vproj(2 * w)
                emit_vproj(2 * w + 1)
            if w % 2 == 1:
                m = (w + 1) // 2
                if m < KC:
                    emit_qkproj(m, 0)
                    emit_qkproj(KC + m, 0)
                if NCH == 4:
                    mb = (w - 1) // 2
                    if mb < KC:
                        emit_qkproj(mb, 1)
                        emit_qkproj(KC + mb, 1)
        # drain the pipeline
        total_units = NBP * HEADS
        while ncc[0] < total_units:
            pump(None)
        emit_proj_upto(len(mt_sizes))

    nc.compile()
    return nc


def _prep_shared(w_qkv, w_proj, rel_pos, rel_pos_index):
    """Host-side input prep shared across cores (weights / bias / identity)."""
    w_qkv = np.asarray(w_qkv, dtype=np.float32)
    w_proj = np.asarray(w_proj, dtype=np.float32)
    rel_pos = np.asarray(rel_pos, dtype=np.float32)
    rel_pos_index = np.asarray(rel_pos_index)

    wqk = w_qkv[:2 * DIM].copy()
    wqk[:DIM] *= SCALE  # fold attention scale into Wq
    wqkT = np.ascontiguousarray(wqk.T)  # [DIM, 2*DIM]
    # interleave q/k column chunks: pair p -> cols [256p:256p+128]=q-chunk p,
    # [256p+128:256(p+1)]=k-chunk p
    wqkT = wqkT.reshape(DIM, 2, KC, 128).transpose(0, 2, 1, 3)
    wqkT = np.ascontiguousarray(wqkT.reshape(DIM, 2 * DIM)).astype(BF16)
    wvT = np.ascontiguousarray(w_qkv[2 * DIM:].T).astype(BF16)
    wpT = np.ascontiguousarray(w_proj.T).astype(BF16)

    bias_full = np.zeros((HEADS, N, N), dtype=np.float32)
    bias_full[:, 1:, 1:] = rel_pos[:, rel_pos_index]
    # exp of the transposed bias ([k, q]) — folded into P by a multiply on
    # device (exp(S+B) = exp(S)*exp(B)); duplication along q happens on-device
    biasT2 = np.ascontiguousarray(np.exp(np.swapaxes(bias_full, 1, 2))).astype(BF16)

    ident = np.eye(128, dtype=BF16)
    return {"wqkT": wqkT, "wvT": wvT, "wpT": wpT, "biasT2": biasT2,
            "ident": ident}


def _prep_core(x, core, bl=BL):
    """Per-core xT: [DIM, bl*N] bf16."""
    xc = np.asarray(x[core * bl:(core + 1) * bl], dtype=np.float32)
    xT = np.ascontiguousarray(xc.reshape(bl * N, DIM).T).astype(BF16)
    return xT


def kernel(x, w_qkv, w_proj, b_proj, rel_pos, rel_pos_index):
    from concourse.bass_utils import run_bass_kernel_spmd

    x = np.asarray(x, dtype=np.float32)
    w_qkv = np.asarray(w_qkv, dtype=np.float32)
    w_proj = np.asarray(w_proj, dtype=np.float32)
    b_proj = np.asarray(b_proj, dtype=np.float32)
    rel_pos = np.asarray(rel_pos, dtype=np.float32)
    rel_pos_index = np.asarray(rel_pos_index)

    if "nc" not in _CACHE:
        _CACHE["nc"] = _build(BL)
    nc = _CACHE["nc"]

    shared = _prep_shared(w_qkv, w_proj, rel_pos, rel_pos_index)
    in_maps = []
    for core in range(NCORES):
        m = dict(shared)
        m["xT"] = _prep_core(x, core)
        in_maps.append(m)

    try:
        y_cores = _run_cached(nc, in_maps)
    except Exception:
        res = run_bass_kernel_spmd(nc, in_maps, core_ids=list(range(NCORES)))
        y_cores = [r["y"] for r in res.results]
    y = np.concatenate(
        [yc.reshape(BL, N, DIM) for yc in y_cores], axis=0
    ).astype(np.float32)
    return y + b_proj[None, None, :]


def _run_cached(nc, in_maps):
    """Execute via a cached jitted shard_map executable (run_bass_kernel_spmd
    re-traces per call; this path pays tracing/lowering only once)."""
    import jax
    from jax.sharding import Mesh, PartitionSpec, NamedSharding
    from jax.experimental.shard_map import shard_map
    from concourse import bass2jax, mybir

    if "exe" not in _CACHE:
        bass2jax.install_neuronx_cc_hook()
        pname = nc.partition_id_tensor.name if nc.partition_id_tensor else None
        in_names, out_names, out_avals, zeros = [], [], [], []
        for alloc in nc.m.functions[0].allocations:
            if not isinstance(alloc, mybir.MemoryLocationSet):
                continue
            name = alloc.memorylocations[0].name
            if alloc.kind == "ExternalInput":
                if name != pname:
                    in_names.append(name)
            elif alloc.kind == "ExternalOutput":
                out_names.append(name)
                shape = tuple(alloc.tensor_shape)
                dtype = mybir.dt.np(alloc.dtype)
                out_avals.append(jax.core.ShapedArray(shape, dtype))
                zeros.append(np.zeros(shape, dtype))
        n_params = len(in_names)
        all_in = in_names + out_names + ([pname] if pname else [])

        def _body(*args):
            operands = list(args)
            if pname is not None:
                operands.append(bass2jax.partition_id_tensor())
            return tuple(bass2jax._bass_exec_p.bind(
                *operands, out_avals=tuple(out_avals), in_names=tuple(all_in),
                out_names=tuple(out_names), lowering_input_output_aliases=(),
                sim_require_finite=True, sim_require_nnan=True, nc=nc))

        devices = jax.devices()[:NCORES]
        mesh = Mesh(np.asarray(devices), ("core",))
        n_outs = len(out_names)
        sharded = jax.jit(
            shard_map(_body, mesh=mesh,
                      in_specs=(PartitionSpec("core"),) * (n_params + n_outs),
                      out_specs=(PartitionSpec("core"),) * n_outs,
                      check_rep=False),
            keep_unused=True,
        )
        sh = NamedSharding(mesh, PartitionSpec("core"))
        zero_dev = [
            jax.device_put(
                np.zeros((NCORES * z.shape[0], *z.shape[1:]), z.dtype), sh)
            for z in zeros
        ]
        _CACHE["exe"] = (sharded, in_names, out_names, zero_dev, sh)

    sharded, in_names, out_names, zero_dev, sh = _CACHE["exe"]
    concat_in = [
        np.concatenate([np.asarray(in_maps[c][nm]) for c in range(NCORES)],
                       axis=0)
        for nm in in_names
    ]
    out = sharded(*[jax.device_put(a, sh) for a in concat_in], *zero_dev)
    yi = out_names.index("y")
    y_all = np.asarray(out[yi])
    rows = y_all.shape[0] // NCORES
    return [y_all[c * rows:(c + 1) * rows] for c in range(NCORES)]
